# revision 2
# baseline (speedup 1.0000x reference)
"""Trainium2 kernel for nn_MeshAutoencoder (vq_codebook).

Strategy: all GEMM-heavy compute (encoder projections, SAGE conv linear
layers, codebook distance matmuls) runs on 8 NeuronCores via a generic
row-sharded matmul program (each core computes a 1/8 slice of rows).
Graph index gather/scatter runs between device launches. The VQ argmin
uses a provably-bounded candidate set (codebook sorted by norm / by the
g-score), verified with an exact Cauchy-Schwarz-style bound; a full
16384-wide fallback path runs if the bound check ever fails.
"""
import sys
import json
import numpy as np

sys.path.insert(0, '/opt/trn_rl_repo')

import concourse.bass as bass
import concourse.mybir as mybir
from concourse.bass_utils import run_bass_kernel_spmd
from concourse.tile import TileContext

# ---- problem constants (hardcoded per contract) ----
DIM = 512
NUM_DISCRETE = 128
DCE = 64
DCB = 192
KCB = 16384
B, NV, NF, E = 2, 10000, 20000, 60000
NCORES = 8

# generic matmul program shape
GM = 40960          # rows, 5120 per core
GK = 1152           # contraction (9 chunks of 128)
GN = 1024           # output cols (2 PSUM chunks of 512)
MPC = GM // NCORES  # rows per core

_MAX_WAITS = 1


def _fix_bir_json(bir: bytes) -> bytes:
    """This walrus build only allows 1 sem-wait per instruction; hoist
    excess waits onto preceding NoOps (semantics preserving)."""
    m = json.loads(bir)
    counter = [0]

    def fresh():
        counter[0] += 1
        return f"I-waitfix-{counter[0]}"

    changed = False
    for f in m.get("functions", []):
        for bb in f.get("blocks", []) or []:
            out = []
            for ins in bb.get("instructions", []):
                si = ins.get("sync_info")
                waits = (si or {}).get("on_wait") or []
                if len(waits) > _MAX_WAITS:
                    excess = waits[:-_MAX_WAITS]
                    keep = waits[-_MAX_WAITS:]
                    for i in range(0, len(excess), _MAX_WAITS):
                        chunk = excess[i:i + _MAX_WAITS]
                        out.append({
                            "debug": ins.get("debug", 0),
                            "engine": ins["engine"],
                            "ins": [], "name": fresh(), "opcode": "NoOp",
                            "outs": [],
                            "sync_info": {"on_update": [], "on_wait": chunk},
                        })
                    si["on_wait"] = keep
                    changed = True
                out.append(ins)
            bb["instructions"] = out
    return json.dumps(m).encode() if changed else bir


def _wrap_fix(nc):
    orig = nc.to_json_bytes
    nc.to_json_bytes = lambda: _fix_bir_json(orig())
    return nc


_PROGRAM = None


def _build_program():
    """Generic row-sharded matmul: C[MPC, GN] = AT.T @ Bmat per core.
    AT: [GK, MPC] (A transposed, this core's row shard), Bmat: [GK, GN].
    fp32 native (exact)."""
    global _PROGRAM
    if _PROGRAM is not None:
        return _PROGRAM
    nc = bass.Bass(num_devices=NCORES)
    at_ext = nc.declare_dram_parameter("AT", [GK, MPC], mybir.dt.float32, isOutput=False)
    b_ext = nc.declare_dram_parameter("Bmat", [GK, GN], mybir.dt.float32, isOutput=False)
    c_ext = nc.declare_dram_parameter("C", [MPC, GN], mybir.dt.float32, isOutput=True)
    KC = GK // 128
    NCH = GN // 512
    MT = MPC // 128
    with TileContext(nc) as tc:
        with tc.tile_pool(name="wpool", bufs=1) as wp, \
             tc.tile_pool(name="apool", bufs=3) as ap_, \
             tc.tile_pool(name="opool", bufs=3) as op_, \
             tc.tile_pool(name="psum", bufs=4, space="PSUM") as ps:
            bt = wp.tile([128, KC, GN], mybir.dt.float32, tag="B")
            nc.sync.dma_start(out=bt[:], in_=b_ext[:].rearrange("(c p) n -> p c n", p=128))
            for mt in range(MT):
                msl = slice(mt * 128, (mt + 1) * 128)
                at_t = ap_.tile([128, KC, 128], mybir.dt.float32, tag="A")
                nc.sync.dma_start(
                    out=at_t[:],
                    in_=at_ext[:, msl].rearrange("(c p) n -> p c n", p=128))
                for nch in range(NCH):
                    nsl = slice(nch * 512, (nch + 1) * 512)
                    p = ps.tile([128, 512], mybir.dt.float32, tag="p")
                    for kc in range(KC):
                        nc.tensor.matmul(
                            out=p[:], lhsT=at_t[:, kc, :], rhs=bt[:, kc, nsl],
                            start=(kc == 0), stop=(kc == KC - 1))
                    o = op_.tile([128, 512], mybir.dt.float32, tag="o")
                    nc.vector.tensor_copy(out=o[:], in_=p[:])
                    nc.sync.dma_start(out=c_ext[msl, nsl], in_=o[:])
    _wrap_fix(nc)
    _PROGRAM = nc
    return nc


def _mm(A, Bmat, n_out):
    """Device matmul: returns A @ Bmat[:, :n_out] computed on 8 cores.
    A: [m, k] fp32 (m<=GM, k<=GK), Bmat: [k, n] (n<=GN)."""
    nc = _build_program()
    m, k = A.shape
    n = Bmat.shape[1]
    assert m <= GM and k <= GK and n <= GN
    AT = np.zeros((GK, GM), np.float32)
    AT[:k, :m] = A.T
    Bp = np.zeros((GK, GN), np.float32)
    Bp[:k, :n] = Bmat
    in_maps = [{"AT": np.ascontiguousarray(AT[:, c * MPC:(c + 1) * MPC]), "Bmat": Bp}
               for c in range(NCORES)]
    res = run_bass_kernel_spmd(nc, in_maps, core_ids=list(range(NCORES)))
    C = np.concatenate([r["C"] for r in res.results], axis=0)
    return C[:m, :n_out]


def _discretize(v):
    t = (v + 1.0) / 2.0 * NUM_DISCRETE - 0.5
    return np.clip(np.round(t), 0, NUM_DISCRETE - 1).astype(np.int64)


def _segsum(vals, idx, nseg):
    """Exact segment sum via sort + reduceat (much faster than np.add.at)."""
    order = np.argsort(idx, kind='stable')
    sv = vals[order]
    si = idx[order]
    boundaries = np.flatnonzero(np.r_[True, si[1:] != si[:-1]])
    sums = np.add.reduceat(sv, boundaries, axis=0)
    out = np.zeros((nseg,) + vals.shape[1:], vals.dtype)
    out[si[boundaries]] = sums
    return out


def kernel(vertices, faces, face_edges, coor_embed, W_in, b_in,
           Wl0, bl0, Wr0, Wl1, bl1, Wr1, W_cb, b_cb, codebook):
    vertices = np.asarray(vertices, np.float32)
    faces = np.asarray(faces).astype(np.int64)
    face_edges = np.asarray(face_edges).astype(np.int64)
    coor_embed = np.asarray(coor_embed, np.float32)
    W_in = np.asarray(W_in, np.float32); b_in = np.asarray(b_in, np.float32)
    Wl0 = np.asarray(Wl0, np.float32); bl0 = np.asarray(bl0, np.float32)
    Wr0 = np.asarray(Wr0, np.float32)
    Wl1 = np.asarray(Wl1, np.float32); bl1 = np.asarray(bl1, np.float32)
    Wr1 = np.asarray(Wr1, np.float32)
    W_cb = np.asarray(W_cb, np.float32); b_cb = np.asarray(b_cb, np.float32)
    codebook = np.asarray(codebook, np.float32)

    b_total = vertices.shape[0]
    out = np.zeros((b_total, NF, 3 * DCB), np.float32)

    # --- codebook candidate prep (host: static weight analysis) ---
    cb_sq = np.sum(codebook.astype(np.float64) * codebook, axis=-1).astype(np.float32)
    cn = np.sqrt(cb_sq)
    order1 = np.argsort(cb_sq, kind='stable')
    C1 = 1024

    # batch both graphs through each GEMM launch together where possible:
    # stack batches along rows (2*20000 = 40000 <= GM).
    disc = _discretize(vertices)                      # [B, NV, 3]
    e_tab = coor_embed[disc].reshape(b_total, NV, 3 * DCE)  # [B, NV, 192]
    emb = np.stack([e_tab[b][faces[b]].reshape(NF, 9 * DCE) for b in range(b_total)])
    emb2 = emb.reshape(b_total * NF, 9 * DCE)

    # x = emb @ W_in + b_in   (bias via ones-column)
    A1 = np.concatenate([emb2, np.ones((emb2.shape[0], 1), np.float32)], axis=1)
    B1 = np.concatenate([W_in, b_in[None, :]], axis=0)
    x = _mm(A1, B1, DIM)                               # [B*NF, 512]

    offset = (np.arange(b_total) * NF)[:, None]
    src = (face_edges[:, 0] + offset).reshape(-1)
    dst = (face_edges[:, 1] + offset).reshape(-1)
    nfb = b_total * NF
    cnt = np.bincount(dst, minlength=nfb).astype(np.float32)
    inv_cnt = 1.0 / np.maximum(cnt, 1.0)

    for (Wl, bl, Wr) in ((Wl0, bl0, Wr0), (Wl1, bl1, Wr1)):
        agg = _segsum(x[src], dst, nfb)
        mean = agg * inv_cnt[:, None]
        Ac = np.concatenate([mean, x, np.ones((nfb, 1), np.float32)], axis=1)
        Bc = np.concatenate([Wl, Wr, bl[None, :]], axis=0)
        x = _mm(Ac, Bc, DIM)

    # fe = x @ W_cb + b_cb
    Af = np.concatenate([x, np.ones((nfb, 1), np.float32)], axis=1)
    Bf = np.concatenate([W_cb, b_cb[None, :]], axis=0)
    fe = _mm(Af, Bf, 3 * DCB)                          # [B*NF, 576]
    fe = fe.reshape(b_total, NF * 3, DCB)

    faces_flat = faces.reshape(b_total, NF * 3)
    quantized = np.zeros((b_total, NV, DCB), np.float32)
    avg = np.zeros((b_total, NV, DCB), np.float32)
    for b in range(b_total):
        num = _segsum(fe[b], faces_flat[b], NV)
        den = np.bincount(faces_flat[b], minlength=NV).astype(np.float32)
        avg[b] = num / np.maximum(den, 1e-5)[:, None]

    # ---- VQ: two quantizers, candidate-set argmin on device ----
    residual = avg.reshape(b_total * NV, DCB).copy()
    rn_all = np.linalg.norm(residual.astype(np.float64), axis=1)

    def vq_round(res, cand_idx):
        """exact argmin over codebook restricted to cand_idx, via device GEMM.
        s_k = 2 r.c_k - |c_k|^2 ; returns global winner indices."""
        nc_cand = len(cand_idx)
        Csub = codebook[cand_idx]                      # [nc, 192]
        Bq = np.concatenate([2.0 * Csub.T, -cb_sq[cand_idx][None, :]], axis=0)  # [193, nc]
        Aq = np.concatenate([res, np.ones((res.shape[0], 1), np.float32)], axis=1)
        s = _mm(Aq, Bq, nc_cand)                       # [nr, nc]
        loc = np.argmax(s, axis=1)
        return cand_idx[loc], np.max(s, axis=1)

    def verify_covered(s_best, rmax, cand_mask):
        """Cauchy-Schwarz: excluded k can win only if 2|r| cn_k - cb_sq_k >= s_best."""
        bound = 2.0 * rmax * cn - cb_sq
        return not np.any(bound[~cand_mask] >= s_best.min() - 1e-4)

    for q in range(2):
        rmax = np.linalg.norm(residual.astype(np.float64), axis=1).max()
        # candidate set: lowest-norm prefix; widen by the g-score when q>0
        if q == 0:
            cand = np.sort(order1[:C1])
        else:
            # residuals cluster near -c_j for the q1 winners j; use the union
            # of g-orders for observed winners
            uniq = np.unique(idx_prev)
            gsets = []
            for j in uniq[:8]:
                g = -2.0 * codebook @ codebook[j] - cb_sq
                gsets.append(np.argsort(-g, kind='stable')[:C1])
            cand = np.unique(np.concatenate(gsets))[:4096]
        mask = np.zeros(KCB, bool)
        mask[cand] = True
        idx, s_best = vq_round(residual, cand)
        if q == 0:
            ok = verify_covered(s_best, rmax, mask)
        else:
            # decomposition bound: s2_k = g^(j)_k + 2 avg.c_k, |2 avg.c_k| <= 2|avg| cn_k
            avn = rn_all.max()
            ok = True
            for j in np.unique(idx_prev):
                g = -2.0 * codebook @ codebook[j] - cb_sq
                sel = idx_prev == j
                lhs = (g + 2.0 * avn * cn)[~mask].max()
                if lhs >= s_best[sel].min() - 1e-4:
                    ok = False
                    break
        if not ok:
            # exact fallback: full codebook in GN-sized column chunks
            best_s = np.full(residual.shape[0], -np.inf, np.float32)
            best_i = np.zeros(residual.shape[0], np.int64)
            for k0 in range(0, KCB, GN):
                ci = np.arange(k0, min(k0 + GN, KCB))
                ii, ss = vq_round(residual, ci)
                upd = ss > best_s
                best_s[upd] = ss[upd]
                best_i[upd] = ii[upd]
            idx = best_i
        idx_prev = idx
        qv = codebook[idx]
        quantized += qv.reshape(b_total, NV, DCB)
        residual -= qv

    for b in range(b_total):
        out[b] = quantized[b][faces_flat[b]].reshape(NF, 3 * DCB)
    return out


# revision 3
# speedup vs baseline: 1.4454x; 1.4454x over previous
"""Trainium2 kernel for nn_MeshAutoencoder (vq_codebook).

Strategy: all GEMM-heavy compute (encoder projections, SAGE conv linear
layers, codebook distance matmuls) runs on 8 NeuronCores via a generic
row-sharded matmul program (each core computes a 1/8 slice of rows).
Graph index gather/scatter runs between device launches. The VQ argmin
uses a provably-bounded candidate set (codebook sorted by norm / by the
g-score), verified with an exact Cauchy-Schwarz-style bound; a full
16384-wide fallback path runs if the bound check ever fails.
"""
import sys
import json
import numpy as np

sys.path.insert(0, '/opt/trn_rl_repo')

import concourse.bass as bass
import concourse.mybir as mybir
from concourse.bass_utils import run_bass_kernel_spmd
from concourse.tile import TileContext

# ---- problem constants (hardcoded per contract) ----
DIM = 512
NUM_DISCRETE = 128
DCE = 64
DCB = 192
KCB = 16384
B, NV, NF, E = 2, 10000, 20000, 60000
NCORES = 8

# generic matmul program shape
GM = 40960          # rows, 5120 per core
GK = 1152           # contraction (9 chunks of 128)
GN = 1024           # output cols (2 PSUM chunks of 512)
MPC = GM // NCORES  # rows per core

_MAX_WAITS = 1


def _fix_bir_json(bir: bytes) -> bytes:
    """This walrus build only allows 1 sem-wait per instruction; hoist
    excess waits onto preceding NoOps (semantics preserving)."""
    m = json.loads(bir)
    counter = [0]

    def fresh():
        counter[0] += 1
        return f"I-waitfix-{counter[0]}"

    changed = False
    for f in m.get("functions", []):
        for bb in f.get("blocks", []) or []:
            out = []
            for ins in bb.get("instructions", []):
                si = ins.get("sync_info")
                waits = (si or {}).get("on_wait") or []
                if len(waits) > _MAX_WAITS:
                    excess = waits[:-_MAX_WAITS]
                    keep = waits[-_MAX_WAITS:]
                    for i in range(0, len(excess), _MAX_WAITS):
                        chunk = excess[i:i + _MAX_WAITS]
                        out.append({
                            "debug": ins.get("debug", 0),
                            "engine": ins["engine"],
                            "ins": [], "name": fresh(), "opcode": "NoOp",
                            "outs": [],
                            "sync_info": {"on_update": [], "on_wait": chunk},
                        })
                    si["on_wait"] = keep
                    changed = True
                out.append(ins)
            bb["instructions"] = out
    return json.dumps(m).encode() if changed else bir


def _wrap_fix(nc):
    orig = nc.to_json_bytes
    nc.to_json_bytes = lambda: _fix_bir_json(orig())
    return nc


_PROGRAMS = {}


def _build_program(KC, NLIST, MT):
    """Row-sharded matmul program for shape: K=128*KC, N=sum(NLIST), M=128*MT*NCORES.
    NLIST = PSUM column chunk sizes (each <=512)."""
    key = (KC, tuple(NLIST), MT)
    if key in _PROGRAMS:
        return _PROGRAMS[key]
    mpc = 128 * MT
    gn = sum(NLIST)
    gk = 128 * KC
    nc = bass.Bass(num_devices=NCORES)
    at_ext = nc.declare_dram_parameter("AT", [gk, mpc], mybir.dt.float32, isOutput=False)
    b_ext = nc.declare_dram_parameter("Bmat", [gk, gn], mybir.dt.float32, isOutput=False)
    c_ext = nc.declare_dram_parameter("C", [mpc, gn], mybir.dt.float32, isOutput=True)
    with TileContext(nc) as tc:
        with tc.tile_pool(name="wpool", bufs=1) as wp, \
             tc.tile_pool(name="apool", bufs=3) as ap_, \
             tc.tile_pool(name="opool", bufs=3) as op_, \
             tc.tile_pool(name="psum", bufs=4, space="PSUM") as ps:
            bt = wp.tile([128, KC, gn], mybir.dt.float32, tag="B")
            nc.sync.dma_start(out=bt[:], in_=b_ext[:].rearrange("(c p) n -> p c n", p=128))
            for mt in range(MT):
                msl = slice(mt * 128, (mt + 1) * 128)
                at_t = ap_.tile([128, KC, 128], mybir.dt.float32, tag="A")
                nc.sync.dma_start(
                    out=at_t[:],
                    in_=at_ext[:, msl].rearrange("(c p) n -> p c n", p=128))
                n0 = 0
                for nsz in NLIST:
                    nsl = slice(n0, n0 + nsz)
                    n0 += nsz
                    p = ps.tile([128, 512], mybir.dt.float32, tag="p")
                    for kc in range(KC):
                        nc.tensor.matmul(
                            out=p[:, :nsz], lhsT=at_t[:, kc, :], rhs=bt[:, kc, nsl],
                            start=(kc == 0), stop=(kc == KC - 1))
                    o = op_.tile([128, 512], mybir.dt.float32, tag="o")
                    nc.vector.tensor_copy(out=o[:, :nsz], in_=p[:, :nsz])
                    nc.sync.dma_start(out=c_ext[msl, nsl], in_=o[:, :nsz])
    _wrap_fix(nc)
    _PROGRAMS[key] = nc
    return nc


def _mm(A, Bmat, n_out):
    """Device matmul on 8 cores: A @ Bmat[:, :n_out]."""
    m, k = A.shape
    n = Bmat.shape[1]
    KC = (k + 127) // 128
    MT = (m + 128 * NCORES - 1) // (128 * NCORES)
    NLIST = []
    left = n
    while left > 0:
        NLIST.append(min(512, left))
        left -= min(512, left)
    nc = _build_program(KC, NLIST, MT)
    mpc = 128 * MT
    gm = mpc * NCORES
    gk = 128 * KC
    AT = np.zeros((gk, gm), np.float32)
    AT[:k, :m] = A.T
    Bp = np.zeros((gk, n), np.float32)
    Bp[:k, :] = Bmat
    in_maps = [{"AT": np.ascontiguousarray(AT[:, c * mpc:(c + 1) * mpc]), "Bmat": Bp}
               for c in range(NCORES)]
    res = run_bass_kernel_spmd(nc, in_maps, core_ids=list(range(NCORES)))
    C = np.concatenate([r["C"] for r in res.results], axis=0)
    return C[:m, :n_out]


def _discretize(v):
    t = (v + 1.0) / 2.0 * NUM_DISCRETE - 0.5
    return np.clip(np.round(t), 0, NUM_DISCRETE - 1).astype(np.int64)


def _segsum(vals, idx, nseg):
    """Exact segment sum via sort + reduceat (much faster than np.add.at)."""
    order = np.argsort(idx, kind='stable')
    sv = vals[order]
    si = idx[order]
    boundaries = np.flatnonzero(np.r_[True, si[1:] != si[:-1]])
    sums = np.add.reduceat(sv, boundaries, axis=0)
    out = np.zeros((nseg,) + vals.shape[1:], vals.dtype)
    out[si[boundaries]] = sums
    return out


def kernel(vertices, faces, face_edges, coor_embed, W_in, b_in,
           Wl0, bl0, Wr0, Wl1, bl1, Wr1, W_cb, b_cb, codebook):
    vertices = np.asarray(vertices, np.float32)
    faces = np.asarray(faces).astype(np.int64)
    face_edges = np.asarray(face_edges).astype(np.int64)
    coor_embed = np.asarray(coor_embed, np.float32)
    W_in = np.asarray(W_in, np.float32); b_in = np.asarray(b_in, np.float32)
    Wl0 = np.asarray(Wl0, np.float32); bl0 = np.asarray(bl0, np.float32)
    Wr0 = np.asarray(Wr0, np.float32)
    Wl1 = np.asarray(Wl1, np.float32); bl1 = np.asarray(bl1, np.float32)
    Wr1 = np.asarray(Wr1, np.float32)
    W_cb = np.asarray(W_cb, np.float32); b_cb = np.asarray(b_cb, np.float32)
    codebook = np.asarray(codebook, np.float32)

    b_total = vertices.shape[0]
    out = np.zeros((b_total, NF, 3 * DCB), np.float32)

    # --- codebook candidate prep (host: static weight analysis) ---
    cb_sq = np.sum(codebook.astype(np.float64) * codebook, axis=-1).astype(np.float32)
    cn = np.sqrt(cb_sq)
    order1 = np.argsort(cb_sq, kind='stable')
    C1 = 1024

    # batch both graphs through each GEMM launch together where possible:
    # stack batches along rows (2*20000 = 40000 <= GM).
    disc = _discretize(vertices)                      # [B, NV, 3]
    e_tab = coor_embed[disc].reshape(b_total, NV, 3 * DCE)  # [B, NV, 192]
    emb = np.stack([e_tab[b][faces[b]].reshape(NF, 9 * DCE) for b in range(b_total)])
    emb2 = emb.reshape(b_total * NF, 9 * DCE)

    # x = emb @ W_in + b_in   (bias via ones-column)
    A1 = np.concatenate([emb2, np.ones((emb2.shape[0], 1), np.float32)], axis=1)
    B1 = np.concatenate([W_in, b_in[None, :]], axis=0)
    x = _mm(A1, B1, DIM)                               # [B*NF, 512]

    offset = (np.arange(b_total) * NF)[:, None]
    src = (face_edges[:, 0] + offset).reshape(-1)
    dst = (face_edges[:, 1] + offset).reshape(-1)
    nfb = b_total * NF
    cnt = np.bincount(dst, minlength=nfb).astype(np.float32)
    inv_cnt = 1.0 / np.maximum(cnt, 1.0)

    for (Wl, bl, Wr) in ((Wl0, bl0, Wr0), (Wl1, bl1, Wr1)):
        agg = _segsum(x[src], dst, nfb)
        mean = agg * inv_cnt[:, None]
        Ac = np.concatenate([mean, x, np.ones((nfb, 1), np.float32)], axis=1)
        Bc = np.concatenate([Wl, Wr, bl[None, :]], axis=0)
        x = _mm(Ac, Bc, DIM)

    # fe = x @ W_cb + b_cb
    Af = np.concatenate([x, np.ones((nfb, 1), np.float32)], axis=1)
    Bf = np.concatenate([W_cb, b_cb[None, :]], axis=0)
    fe = _mm(Af, Bf, 3 * DCB)                          # [B*NF, 576]
    fe = fe.reshape(b_total, NF * 3, DCB)

    faces_flat = faces.reshape(b_total, NF * 3)
    quantized = np.zeros((b_total, NV, DCB), np.float32)
    avg = np.zeros((b_total, NV, DCB), np.float32)
    for b in range(b_total):
        num = _segsum(fe[b], faces_flat[b], NV)
        den = np.bincount(faces_flat[b], minlength=NV).astype(np.float32)
        avg[b] = num / np.maximum(den, 1e-5)[:, None]

    # ---- VQ: two quantizers, candidate-set argmin on device ----
    residual = avg.reshape(b_total * NV, DCB).copy()
    rn_all = np.linalg.norm(residual.astype(np.float64), axis=1)

    def vq_round(res, cand_idx):
        """exact argmin over codebook restricted to cand_idx, via device GEMM.
        s_k = 2 r.c_k - |c_k|^2 ; returns global winner indices."""
        nc_cand = len(cand_idx)
        Csub = codebook[cand_idx]                      # [nc, 192]
        Bq = np.concatenate([2.0 * Csub.T, -cb_sq[cand_idx][None, :]], axis=0)  # [193, nc]
        Aq = np.concatenate([res, np.ones((res.shape[0], 1), np.float32)], axis=1)
        s = _mm(Aq, Bq, nc_cand)                       # [nr, nc]
        loc = np.argmax(s, axis=1)
        return cand_idx[loc], np.max(s, axis=1)

    def verify_covered(s_best, rmax, cand_mask):
        """Cauchy-Schwarz: excluded k can win only if 2|r| cn_k - cb_sq_k >= s_best."""
        bound = 2.0 * rmax * cn - cb_sq
        return not np.any(bound[~cand_mask] >= s_best.min() - 1e-4)

    for q in range(2):
        rmax = np.linalg.norm(residual.astype(np.float64), axis=1).max()
        # candidate set: lowest-norm prefix; widen by the g-score when q>0
        if q == 0:
            cand = np.sort(order1[:C1])
        else:
            # residuals cluster near -c_j for the q1 winners j; use the union
            # of g-orders for observed winners
            uniq = np.unique(idx_prev)
            gsets = []
            for j in uniq[:8]:
                g = -2.0 * codebook @ codebook[j] - cb_sq
                gsets.append(np.argsort(-g, kind='stable')[:C1])
            cand = np.unique(np.concatenate(gsets))[:4096]
        mask = np.zeros(KCB, bool)
        mask[cand] = True
        idx, s_best = vq_round(residual, cand)
        if q == 0:
            ok = verify_covered(s_best, rmax, mask)
        else:
            # decomposition bound: s2_k = g^(j)_k + 2 avg.c_k, |2 avg.c_k| <= 2|avg| cn_k
            avn = rn_all.max()
            ok = True
            for j in np.unique(idx_prev):
                g = -2.0 * codebook @ codebook[j] - cb_sq
                sel = idx_prev == j
                lhs = (g + 2.0 * avn * cn)[~mask].max()
                if lhs >= s_best[sel].min() - 1e-4:
                    ok = False
                    break
        if not ok:
            # exact fallback: full codebook in GN-sized column chunks
            best_s = np.full(residual.shape[0], -np.inf, np.float32)
            best_i = np.zeros(residual.shape[0], np.int64)
            for k0 in range(0, KCB, GN):
                ci = np.arange(k0, min(k0 + GN, KCB))
                ii, ss = vq_round(residual, ci)
                upd = ss > best_s
                best_s[upd] = ss[upd]
                best_i[upd] = ii[upd]
            idx = best_i
        idx_prev = idx
        qv = codebook[idx]
        quantized += qv.reshape(b_total, NV, DCB)
        residual -= qv

    for b in range(b_total):
        out[b] = quantized[b][faces_flat[b]].reshape(NF, 3 * DCB)
    return out
